# revision 4
# baseline (speedup 1.0000x reference)
"""Trainium2 Bass kernel for nn_Attention_19430432047265.

Multi-head attention block (B=2, S=2048, D=4096, 32 heads, head_dim=128) with
RoPE, KV-cache update, causal softmax, and output projection — tensor-parallel
over heads across 8 NeuronCores (4 heads/core), wo row-sharded with host-side
reduction of partial outputs.

Per-core dataflow (all matmuls in fp32r — full-rate PE, ~fp32 precision):
  Pass Q/K/V: natural-layout projections from host-pretransposed xT
              (lhsT = xT tile [128k,128s], rhs = w [128k,512]), RoPE applied
              in natural layout via strided DVE ops, Q/K PE-transposed to
              [d, s] layout and spilled to DRAM.
  Attention:  per (batch, head): scoresT[kp,q] = KT_tile.T @ QT (contraction
              over d=128 in one matmul), unstable softmax (exp on ACT with the
              1/sqrt(d) scale folded in; column sums via an all-ones stationary
              matmul which also broadcasts the sums to all 128 partitions;
              masked tiles skipped / diagonal tiles get an additive -1e4 mask
              before exp), ctxT[dv,q] += V_tile.T @ expT accumulated in PSUM,
              then ctxT *= 1/sums.
  Output:     out[s,:] = sum_h ctxT_h[:, s].T @ wo_rows_h — partial over this
              core's heads; host sums the 8 partials.
"""
import sys
sys.path.insert(0, "/opt/trn_rl_repo")
import numpy as np

import concourse.bass as bass
import concourse.mybir as mybir
import concourse.tile as tile
from concourse import bacc

B, S, DIM, N_HEADS, HEAD_DIM = 2, 2048, 4096, 32, 128
BS = B * S                      # 4096 flattened rows
N_CORES = 8
HPC = N_HEADS // N_CORES        # 4 heads per core
HD_C = HPC * HEAD_DIM           # 512 cols per core
P = 128
NT = 512                        # matmul moving free dim
KT = DIM // P                   # 32 k-tiles for projections
MT = BS // P                    # 32 s-tiles
SCALE = float(1.0 / np.sqrt(HEAD_DIM))

F32 = mybir.dt.float32
F32R = mybir.dt.float32r
EXP = mybir.ActivationFunctionType.Exp

_runner_cache: dict = {}


def _build(mode: str, loop_r: int = 1):
    """mode: 'causal' | 'zero' | 'general'."""
    nc = bacc.Bacc("TRN2", target_bir_lowering=False, debug=False,
                   num_devices=N_CORES)
    xT = nc.declare_dram_parameter("xT", [DIM, BS], F32R, isOutput=False)
    wq = nc.declare_dram_parameter("wq", [DIM, HD_C], F32R, isOutput=False)
    wk = nc.declare_dram_parameter("wk", [DIM, HD_C], F32R, isOutput=False)
    wv = nc.declare_dram_parameter("wv", [DIM, HD_C], F32R, isOutput=False)
    wo = nc.declare_dram_parameter("wo", [HD_C, DIM], F32R, isOutput=False)
    ropeC = nc.declare_dram_parameter("ropeC", [S, HEAD_DIM], F32, isOutput=False)
    ropeS = nc.declare_dram_parameter("ropeS", [S, HEAD_DIM], F32, isOutput=False)
    ident = nc.declare_dram_parameter("ident", [P, P], F32R, isOutput=False)
    ones = nc.declare_dram_parameter("ones", [P, P], F32R, isOutput=False)
    if mode == "causal":
        maskdiag = nc.declare_dram_parameter("maskdiag", [P, 4 * NT], F32,
                                             isOutput=False)
    elif mode == "general":
        maskT = nc.declare_dram_parameter("maskT", [S, S], F32, isOutput=False)
    knat = nc.declare_dram_parameter("knat", [BS, HD_C], F32R, isOutput=True)
    vnat = nc.declare_dram_parameter("vnat", [BS, HD_C], F32R, isOutput=True)
    pout = nc.declare_dram_parameter("pout", [BS, DIM], F32, isOutput=True)
    qt_d = nc.dram_tensor("qt_d", [HPC, P, BS], F32R)
    kt_d = nc.dram_tensor("kt_d", [HPC, P, BS], F32R)

    def maybe_loop(tc, fn):
        if loop_r > 1:
            with tc.For_i(0, loop_r, 1):
                fn()
        else:
            fn()

    with tile.TileContext(nc) as tc:
        with tc.tile_pool(name="const", bufs=1) as constp:
            id_sb = constp.tile([P, P], F32R, tag="id")
            nc.sync.dma_start(id_sb[:], ident[:])
            ones_sb = constp.tile([P, P], F32R, tag="ones")
            nc.sync.dma_start(ones_sb[:], ones[:])
            if mode == "causal":
                md_sb = constp.tile([P, 4 * NT], F32, tag="md")
                nc.sync.dma_start(md_sb[:], maskdiag[:])

            # ---- projection passes -------------------------------------
            for pname, w_h, tdst, out_nat in (
                    ("q", wq, qt_d, None),
                    ("k", wk, kt_d, knat),
                    ("v", wv, None, vnat)):
                with tc.tile_pool(name=f"w_{pname}", bufs=1) as wp, \
                     tc.tile_pool(name=f"xt_{pname}", bufs=2) as xtp, \
                     tc.tile_pool(name=f"tab_{pname}", bufs=3) as tabp, \
                     tc.tile_pool(name=f"rp_{pname}", bufs=6) as rpp, \
                     tc.tile_pool(name=f"stg_{pname}", bufs=6) as stgp, \
                     tc.tile_pool(name=f"psA_{pname}", bufs=2, space="PSUM") as psA, \
                     tc.tile_pool(name=f"psT_{pname}", bufs=2, space="PSUM") as psT:

                    def pass_body(pname=pname, w_h=w_h, tdst=tdst,
                                  out_nat=out_nat, wp=wp, xtp=xtp, tabp=tabp,
                                  rpp=rpp, stgp=stgp, psA=psA, psT=psT):
                        w_sb = wp.tile([P, KT * NT], F32R, tag="w")
                        nc.sync.dma_start(
                            w_sb[:].rearrange("p (ko n) -> p ko n", ko=KT),
                            w_h.rearrange("(ko p) n -> p ko n", p=P))
                        w3 = w_sb[:].rearrange("p (ko n) -> p ko n", ko=KT)

                        def proj_tile(m, x3, mi):
                            rows = slice(m * P, (m + 1) * P)
                            ps = psA.tile([P, NT], F32, tag="ps")
                            for k in range(KT):
                                nc.tensor.matmul(
                                    ps[:], x3[:, k, mi * P:(mi + 1) * P],
                                    w3[:, k, :],
                                    start=(k == 0), stop=(k == KT - 1))
                            if pname == "v":
                                vs = rpp.tile([P, NT], F32R, tag="ev")
                                nc.vector.tensor_copy(vs[:], ps[:])
                                nc.sync.dma_start(out_nat[rows, :], vs[:])
                                return
                            # RoPE in natural layout
                            srow = (m % (S // P)) * P
                            c_sb = tabp.tile([P, HEAD_DIM], F32, tag="c")
                            nc.sync.dma_start(c_sb[:], ropeC[srow:srow + P, :])
                            s_sb = tabp.tile([P, HEAD_DIM], F32, tag="s")
                            nc.sync.dma_start(s_sb[:], ropeS[srow:srow + P, :])
                            ps3 = ps[:].rearrange("p (hb d) -> p hb d", hb=HPC)
                            sw = rpp.tile([P, NT], F32, tag="sw")
                            sw3 = sw[:].rearrange("p (hb d) -> p hb d", hb=HPC)
                            nc.vector.tensor_copy(sw3[:, :, 0::2], ps3[:, :, 1::2])
                            nc.vector.tensor_copy(sw3[:, :, 1::2], ps3[:, :, 0::2])
                            cb = c_sb[:, None, :].broadcast_to((P, HPC, HEAD_DIM))
                            sb_ = s_sb[:, None, :].broadcast_to((P, HPC, HEAD_DIM))
                            tmp = rpp.tile([P, NT], F32, tag="tmp")
                            tmp3 = tmp[:].rearrange("p (hb d) -> p hb d", hb=HPC)
                            nc.vector.tensor_mul(tmp3[:], ps3[:], cb)
                            nc.vector.tensor_mul(sw3[:], sw3[:], sb_)
                            rp = rpp.tile([P, NT], F32R, tag="rp")
                            nc.vector.tensor_add(rp[:], tmp[:], sw[:])
                            if out_nat is not None:
                                nc.sync.dma_start(out_nat[rows, :], rp[:])
                            for hb in range(HPC):
                                tp = psT.tile([P, P], F32, tag="tp")
                                nc.tensor.transpose(
                                    tp[:].bitcast(F32R),
                                    rp[:, hb * P:(hb + 1) * P], id_sb[:])
                                stg = stgp.tile([P, P], F32R, tag="stg")
                                nc.vector.tensor_copy(stg[:], tp[:])
                                nc.sync.dma_start(tdst[hb, :, rows], stg[:])

                        MG = 2      # m-tiles per xT load: 1 KiB DMA runs
                        for mg in range(MT // MG):
                            grows = slice(mg * MG * P, (mg + 1) * MG * P)
                            xt_sb = xtp.tile([P, KT * MG * P], F32R, tag="xt")
                            nc.sync.dma_start(
                                xt_sb[:].rearrange("p (ko s) -> p ko s", ko=KT),
                                xT[:, grows].rearrange("(ko p) s -> p ko s", p=P))
                            x3 = xt_sb[:].rearrange("p (ko s) -> p ko s", ko=KT)
                            for mi in range(MG):
                                proj_tile(mg * MG + mi, x3, mi)
                    maybe_loop(tc, pass_body)

            # ---- attention + output projection -------------------------
            with tc.tile_pool(name="qt", bufs=2) as qtp, \
                 tc.tile_pool(name="kt", bufs=2) as ktp, \
                 tc.tile_pool(name="vat", bufs=2) as vatp, \
                 tc.tile_pool(name="ex", bufs=18) as exp_, \
                 tc.tile_pool(name="ctx", bufs=1) as ctxp, \
                 tc.tile_pool(name="rec", bufs=3) as recp, \
                 tc.tile_pool(name="wo", bufs=2) as wop, \
                 tc.tile_pool(name="ob", bufs=3) as obp, \
                 tc.tile_pool(name="mt", bufs=3) as mtp, \
                 tc.tile_pool(name="psS", bufs=2, space="PSUM") as psS, \
                 tc.tile_pool(name="psU", bufs=2, space="PSUM") as psU, \
                 tc.tile_pool(name="psC", bufs=2, space="PSUM") as psC, \
                 tc.tile_pool(name="psO", bufs=2, space="PSUM") as psO:

                def attn_body():
                    QCT = S // NT           # 4 q-chunks per batch
                    KTT = S // P            # 16 kp-tiles per batch
                    for b in range(B):
                        ctx_tiles = []
                        for hb in range(HPC):
                            bs0 = b * S
                            qt_sb = qtp.tile([P, S], F32R, tag="qt")
                            nc.sync.dma_start(qt_sb[:], qt_d[hb, :, bs0:bs0 + S])
                            kt_sb = ktp.tile([P, S], F32R, tag="kt")
                            nc.sync.dma_start(kt_sb[:], kt_d[hb, :, bs0:bs0 + S])
                            v_sb = vatp.tile([P, KTT * P], F32R, tag="v")
                            nc.sync.dma_start(
                                v_sb[:].rearrange("p (ko d) -> p ko d", ko=KTT),
                                vnat[bs0:bs0 + S, hb * P:(hb + 1) * P]
                                .rearrange("(ko p) d -> p ko d", p=P))
                            v3 = v_sb[:].rearrange("p (ko d) -> p ko d", ko=KTT)
                            ctx_sb = ctxp.tile([P, S], F32R, tag=f"ctx{hb}")
                            ctx_tiles.append(ctx_sb)
                            for qc in range(QCT):
                                kt_hi = 4 * qc + 4 if mode == "causal" else KTT
                                sum_ps = psU.tile([P, NT], F32, tag="sum")
                                ctx_ps = psC.tile([P, NT], F32, tag="ctx")
                                for k in range(kt_hi):
                                    sc = psS.tile([P, NT], F32, tag="sc")
                                    nc.tensor.matmul(
                                        sc[:], kt_sb[:, k * P:(k + 1) * P],
                                        qt_sb[:, qc * NT:(qc + 1) * NT],
                                        start=True, stop=True)
                                    if mode == "causal" and k >= 4 * qc:
                                        o = k - 4 * qc
                                        nc.vector.tensor_add(
                                            sc[:], sc[:],
                                            md_sb[:, o * NT:(o + 1) * NT])
                                    elif mode == "general":
                                        mt_sb = mtp.tile([P, NT], F32, tag="mt")
                                        nc.sync.dma_start(
                                            mt_sb[:],
                                            maskT[k * P:(k + 1) * P,
                                                  qc * NT:(qc + 1) * NT])
                                        nc.vector.tensor_add(sc[:], sc[:], mt_sb[:])
                                    ex = exp_.tile([P, NT], F32R, tag="ex")
                                    nc.scalar.activation(ex[:], sc[:], EXP,
                                                         scale=SCALE)
                                    nc.tensor.matmul(sum_ps[:], ones_sb[:], ex[:],
                                                     start=(k == 0),
                                                     stop=(k == kt_hi - 1))
                                    nc.tensor.matmul(ctx_ps[:], v3[:, k, :], ex[:],
                                                     start=(k == 0),
                                                     stop=(k == kt_hi - 1))
                                rec = recp.tile([P, NT], F32, tag="rec")
                                nc.vector.reciprocal(rec[:], sum_ps[:])
                                nc.vector.tensor_mul(
                                    ctx_sb[:, qc * NT:(qc + 1) * NT],
                                    ctx_ps[:], rec[:])
                        # wo for batch b
                        wo3_d = wo.rearrange("(ko p) n -> p ko n", p=P)
                        for n in range(DIM // NT):
                            won = wop.tile([P, HPC * NT], F32R, tag="won")
                            nc.sync.dma_start(
                                won[:].rearrange("p (ko n) -> p ko n", ko=HPC),
                                wo3_d[:, :, n * NT:(n + 1) * NT])
                            w3 = won[:].rearrange("p (ko n) -> p ko n", ko=HPC)
                            for m in range(S // P):
                                op = psO.tile([P, NT], F32, tag="op")
                                for h in range(HPC):
                                    nc.tensor.matmul(
                                        op[:],
                                        ctx_tiles[h][:, m * P:(m + 1) * P],
                                        w3[:, h, :],
                                        start=(h == 0), stop=(h == HPC - 1))
                                ob = obp.tile([P, NT], F32, tag="ob")
                                nc.vector.tensor_copy(ob[:], op[:])
                                nc.sync.dma_start(
                                    pout[b * S + m * P: b * S + (m + 1) * P,
                                         n * NT:(n + 1) * NT], ob[:])
                maybe_loop(tc, attn_body)
    nc.finalize()
    return nc


class _Runner:
    """SPMD executor via PJRT/axon, inputs kept on device between runs."""

    def __init__(self, nc, n_cores=N_CORES):
        import jax
        from jax.sharding import Mesh, PartitionSpec, NamedSharding
        from jax.experimental.shard_map import shard_map
        from concourse.bass2jax import (_bass_exec_p, install_neuronx_cc_hook,
                                        partition_id_tensor)
        install_neuronx_cc_hook()
        self.jax = jax
        self.n_cores = n_cores
        pname = nc.partition_id_tensor.name if nc.partition_id_tensor else None
        in_names, out_names, out_avals, zero_outs = [], [], [], []
        for alloc in nc.m.functions[0].allocations:
            if not isinstance(alloc, mybir.MemoryLocationSet):
                continue
            name = alloc.memorylocations[0].name
            if alloc.kind == "ExternalInput":
                if name != pname:
                    in_names.append(name)
            elif alloc.kind == "ExternalOutput":
                out_names.append(name)
                shape = tuple(alloc.tensor_shape)
                dtype = mybir.dt.np(alloc.dtype)
                out_avals.append(jax.core.ShapedArray(shape, dtype))
                zero_outs.append(np.zeros(shape, dtype))
        self.in_names, self.out_names = in_names, out_names
        self.out_avals, self.zero_outs = out_avals, zero_outs
        n_params = len(in_names)
        all_in = in_names + out_names + ([pname] if pname else [])

        def _body(*args):
            operands = list(args)
            if pname is not None:
                operands.append(partition_id_tensor())
            return tuple(_bass_exec_p.bind(
                *operands, out_avals=tuple(out_avals), in_names=tuple(all_in),
                out_names=tuple(out_names), lowering_input_output_aliases=(),
                sim_require_finite=True, sim_require_nnan=True, nc=nc))

        devices = jax.devices()[:n_cores]
        mesh = Mesh(np.asarray(devices), ("core",))
        specs_in = (PartitionSpec("core"),) * (n_params + len(out_names))
        specs_out = (PartitionSpec("core"),) * len(out_names)
        self.fn = jax.jit(
            shard_map(_body, mesh=mesh, in_specs=specs_in,
                      out_specs=specs_out, check_rep=False),
            donate_argnums=tuple(range(n_params, n_params + len(out_names))),
            keep_unused=True)
        self.sharding = NamedSharding(mesh, PartitionSpec("core"))
        self._dev_in = None

    def put_inputs(self, in_maps):
        jax = self.jax
        concat = [np.concatenate([np.asarray(in_maps[c][nm])
                                  for c in range(self.n_cores)], axis=0)
                  for nm in self.in_names]
        self._dev_in = [jax.device_put(a, self.sharding) for a in concat]
        for a in self._dev_in:
            a.block_until_ready()

    def run(self):
        import time
        jax = self.jax
        zs = [jax.device_put(
            np.zeros((self.n_cores * z.shape[0], *z.shape[1:]), z.dtype),
            self.sharding) for z in self.zero_outs]
        for z in zs:
            z.block_until_ready()
        t0 = time.perf_counter()
        outs = self.fn(*self._dev_in, *zs)
        for o in outs:
            o.block_until_ready()
        dt = time.perf_counter() - t0
        res = []
        for c in range(self.n_cores):
            d = {nm: np.asarray(outs[i]).reshape(self.n_cores,
                                                 *self.out_avals[i].shape)[c]
                 for i, nm in enumerate(self.out_names)}
            res.append(d)
        return res, dt


def _detect_mode(mask: np.ndarray) -> str:
    m2 = np.asarray(mask).reshape(S, S)
    if not np.any(m2):
        return "zero"
    causal = np.triu(np.full((S, S), -np.inf, dtype=np.float32), k=1)
    if np.array_equal(m2, causal):
        return "causal"
    return "general"


def _host_prep(x, freqs_cis, mask, mode):
    """Build per-core input maps."""
    xf = np.ascontiguousarray(np.asarray(x, dtype=np.float32).reshape(BS, DIM))
    xT = np.ascontiguousarray(xf.T)
    f = np.asarray(freqs_cis, dtype=np.float32)          # (S, 64, 2)
    cos, sin = f[..., 0], f[..., 1]                       # (S, 64)
    ropeC = np.repeat(cos, 2, axis=1).astype(np.float32)  # (S, 128)
    ropeS = np.empty((S, HEAD_DIM), np.float32)
    ropeS[:, 0::2] = -sin
    ropeS[:, 1::2] = sin
    ident = np.eye(P, dtype=np.float32)
    ones = np.ones((P, P), np.float32)
    extra = {}
    if mode == "causal":
        md = np.zeros((P, 4 * NT), np.float32)
        for o in range(4):
            kp = np.arange(P)[:, None] + o * P
            q = np.arange(NT)[None, :]
            md[:, o * NT:(o + 1) * NT] = np.where(kp > q, -1e4, 0.0)
        extra["maskdiag"] = md
    elif mode == "general":
        m2 = np.asarray(mask, dtype=np.float32).reshape(S, S)
        extra["maskT"] = np.maximum(m2.T / SCALE, -1e4).astype(np.float32)
    wqf = np.asarray(_host_prep.wq, dtype=np.float32)
    wkf = np.asarray(_host_prep.wk, dtype=np.float32)
    wvf = np.asarray(_host_prep.wv, dtype=np.float32)
    wof = np.asarray(_host_prep.wo, dtype=np.float32)
    in_maps = []
    for c in range(N_CORES):
        cols = slice(c * HD_C, (c + 1) * HD_C)
        in_maps.append({
            "xT": xT, "wq": np.ascontiguousarray(wqf[:, cols]),
            "wk": np.ascontiguousarray(wkf[:, cols]),
            "wv": np.ascontiguousarray(wvf[:, cols]),
            "wo": np.ascontiguousarray(wof[cols, :]),
            "ropeC": ropeC, "ropeS": ropeS, "ident": ident, "ones": ones,
            **extra,
        })
    return in_maps


def _get_runner(mode, loop_r=1):
    key = (mode, loop_r)
    if key not in _runner_cache:
        nc = _build(mode, loop_r)
        _runner_cache[key] = _Runner(nc)
    return _runner_cache[key]


def _assemble(results, hidden_state):
    out = np.zeros((BS, DIM), np.float32)
    for c in range(N_CORES):
        out += results[c]["pout"]
    out = out.reshape(B, S, DIM)
    nh = np.array(hidden_state, dtype=np.float32, copy=True)
    for c in range(N_CORES):
        h0 = c * HPC
        k = results[c]["knat"].reshape(B, S, HPC, HEAD_DIM)
        v = results[c]["vnat"].reshape(B, S, HPC, HEAD_DIM)
        nh[0, :B, :, h0:h0 + HPC, :] = k
        nh[1, :B, :, h0:h0 + HPC, :] = v
    return out, nh


def kernel(x, start_pos, freqs_cis, mask, hidden_state, wq, wk, wv, wo):
    assert int(start_pos) == 0, "kernel specialized for start_pos=0 prefill"
    mode = _detect_mode(mask)
    _host_prep.wq, _host_prep.wk, _host_prep.wv, _host_prep.wo = wq, wk, wv, wo
    in_maps = _host_prep(x, freqs_cis, mask, mode)
    runner = _get_runner(mode, loop_r=1)
    runner.put_inputs(in_maps)
    results, _ = runner.run()
    return _assemble(results, hidden_state)


# revision 5
# speedup vs baseline: 2.6656x; 2.6656x over previous
"""Trainium2 Bass kernel for nn_Attention_19430432047265.

Multi-head attention block (B=2, S=2048, D=4096, 32 heads, head_dim=128) with
RoPE, KV-cache update, causal softmax, and output projection — tensor-parallel
over heads across 8 NeuronCores (4 heads/core), wo row-sharded with host-side
reduction of partial outputs.

Per-core dataflow (all matmuls in fp32r — full-rate PE, ~fp32 precision):
  Pass Q/K/V: natural-layout projections from host-pretransposed xT
              (lhsT = xT tile [128k,128s], rhs = w [128k,512]), RoPE applied
              in natural layout via strided DVE ops, Q/K PE-transposed to
              [d, s] layout and spilled to DRAM.
  Attention:  per (batch, head): scoresT[kp,q] = KT_tile.T @ QT (contraction
              over d=128 in one matmul), unstable softmax (exp on ACT with the
              1/sqrt(d) scale folded in; column sums via an all-ones stationary
              matmul which also broadcasts the sums to all 128 partitions;
              masked tiles skipped / diagonal tiles get an additive -1e4 mask
              before exp), ctxT[dv,q] += V_tile.T @ expT accumulated in PSUM,
              then ctxT *= 1/sums.
  Output:     out[s,:] = sum_h ctxT_h[:, s].T @ wo_rows_h — partial over this
              core's heads; host sums the 8 partials.
"""
import sys
sys.path.insert(0, "/opt/trn_rl_repo")
import numpy as np

import concourse.bass as bass
import concourse.mybir as mybir
import concourse.tile as tile
from concourse import bacc

B, S, DIM, N_HEADS, HEAD_DIM = 2, 2048, 4096, 32, 128
BS = B * S                      # 4096 flattened rows
N_CORES = 8
HPC = N_HEADS // N_CORES        # 4 heads per core
HD_C = HPC * HEAD_DIM           # 512 cols per core
P = 128
NT = 512                        # matmul moving free dim
KT = DIM // P                   # 32 k-tiles for projections
MT = BS // P                    # 32 s-tiles
SCALE = float(1.0 / np.sqrt(HEAD_DIM))

F32 = mybir.dt.float32
F32R = mybir.dt.float32r
EXP = mybir.ActivationFunctionType.Exp

_runner_cache: dict = {}


def _build(mode: str, loop_r: int = 1):
    """mode: 'causal' | 'zero' | 'general'."""
    nc = bacc.Bacc("TRN2", target_bir_lowering=False, debug=False,
                   num_devices=N_CORES)
    xT = nc.declare_dram_parameter("xT", [DIM, BS], F32R, isOutput=False)
    wq = nc.declare_dram_parameter("wq", [DIM, HD_C], F32R, isOutput=False)
    wk = nc.declare_dram_parameter("wk", [DIM, HD_C], F32R, isOutput=False)
    wv = nc.declare_dram_parameter("wv", [DIM, HD_C], F32R, isOutput=False)
    wo = nc.declare_dram_parameter("wo", [HD_C, DIM], F32R, isOutput=False)
    ropeC = nc.declare_dram_parameter("ropeC", [S, HEAD_DIM], F32, isOutput=False)
    ropeS = nc.declare_dram_parameter("ropeS", [S, HEAD_DIM], F32, isOutput=False)
    ident = nc.declare_dram_parameter("ident", [P, P], F32R, isOutput=False)
    ones = nc.declare_dram_parameter("ones", [P, P], F32R, isOutput=False)
    if mode == "causal":
        maskdiag = nc.declare_dram_parameter("maskdiag", [P, 4 * NT], F32,
                                             isOutput=False)
    elif mode == "general":
        maskT = nc.declare_dram_parameter("maskT", [S, S], F32, isOutput=False)
    knat = nc.declare_dram_parameter("knat", [BS, HD_C], F32R, isOutput=True)
    vnat = nc.declare_dram_parameter("vnat", [BS, HD_C], F32R, isOutput=True)
    pout = nc.declare_dram_parameter("pout", [BS, DIM], F32, isOutput=True)
    qt_d = nc.dram_tensor("qt_d", [HPC, P, BS], F32R)
    kt_d = nc.dram_tensor("kt_d", [HPC, P, BS], F32R)

    def maybe_loop(tc, fn):
        if loop_r > 1:
            with tc.For_i(0, loop_r, 1):
                fn()
        else:
            fn()

    with tile.TileContext(nc) as tc:
        with tc.tile_pool(name="const", bufs=1) as constp:
            id_sb = constp.tile([P, P], F32R, tag="id")
            nc.sync.dma_start(id_sb[:], ident[:])
            ones_sb = constp.tile([P, P], F32R, tag="ones")
            nc.sync.dma_start(ones_sb[:], ones[:])
            if mode == "causal":
                md_sb = constp.tile([P, 4 * NT], F32, tag="md")
                nc.sync.dma_start(md_sb[:], maskdiag[:])

            # ---- projection passes -------------------------------------
            for pname, w_h, tdst, out_nat in (
                    ("q", wq, qt_d, None),
                    ("k", wk, kt_d, knat),
                    ("v", wv, None, vnat)):
                with tc.tile_pool(name=f"w_{pname}", bufs=1) as wp, \
                     tc.tile_pool(name=f"xt_{pname}", bufs=2) as xtp, \
                     tc.tile_pool(name=f"tab_{pname}", bufs=3) as tabp, \
                     tc.tile_pool(name=f"rp_{pname}", bufs=6) as rpp, \
                     tc.tile_pool(name=f"stg_{pname}", bufs=6) as stgp, \
                     tc.tile_pool(name=f"psA_{pname}", bufs=2, space="PSUM") as psA, \
                     tc.tile_pool(name=f"psT_{pname}", bufs=2, space="PSUM") as psT:

                    def pass_body(pname=pname, w_h=w_h, tdst=tdst,
                                  out_nat=out_nat, wp=wp, xtp=xtp, tabp=tabp,
                                  rpp=rpp, stgp=stgp, psA=psA, psT=psT):
                        w_sb = wp.tile([P, KT * NT], F32R, tag="w")
                        nc.sync.dma_start(
                            w_sb[:].rearrange("p (ko n) -> p ko n", ko=KT),
                            w_h.rearrange("(ko p) n -> p ko n", p=P))
                        w3 = w_sb[:].rearrange("p (ko n) -> p ko n", ko=KT)

                        def proj_tile(m, x3, mi):
                            rows = slice(m * P, (m + 1) * P)
                            ps = psA.tile([P, NT], F32, tag="ps")
                            for k in range(KT):
                                nc.tensor.matmul(
                                    ps[:], x3[:, k, mi * P:(mi + 1) * P],
                                    w3[:, k, :],
                                    start=(k == 0), stop=(k == KT - 1))
                            if pname == "v":
                                vs = rpp.tile([P, NT], F32R, tag="ev")
                                nc.vector.tensor_copy(vs[:], ps[:])
                                nc.sync.dma_start(out_nat[rows, :], vs[:])
                                return
                            # RoPE in natural layout
                            srow = (m % (S // P)) * P
                            c_sb = tabp.tile([P, HEAD_DIM], F32, tag="c")
                            nc.sync.dma_start(c_sb[:], ropeC[srow:srow + P, :])
                            s_sb = tabp.tile([P, HEAD_DIM], F32, tag="s")
                            nc.sync.dma_start(s_sb[:], ropeS[srow:srow + P, :])
                            ps3 = ps[:].rearrange("p (hb d) -> p hb d", hb=HPC)
                            sw = rpp.tile([P, NT], F32, tag="sw")
                            sw3 = sw[:].rearrange("p (hb d) -> p hb d", hb=HPC)
                            nc.vector.tensor_copy(sw3[:, :, 0::2], ps3[:, :, 1::2])
                            nc.vector.tensor_copy(sw3[:, :, 1::2], ps3[:, :, 0::2])
                            cb = c_sb[:, None, :].broadcast_to((P, HPC, HEAD_DIM))
                            sb_ = s_sb[:, None, :].broadcast_to((P, HPC, HEAD_DIM))
                            tmp = rpp.tile([P, NT], F32, tag="tmp")
                            tmp3 = tmp[:].rearrange("p (hb d) -> p hb d", hb=HPC)
                            nc.vector.tensor_mul(tmp3[:], ps3[:], cb)
                            nc.vector.tensor_mul(sw3[:], sw3[:], sb_)
                            rp = rpp.tile([P, NT], F32R, tag="rp")
                            nc.vector.tensor_add(rp[:], tmp[:], sw[:])
                            if out_nat is not None:
                                nc.sync.dma_start(out_nat[rows, :], rp[:])
                            for hb in range(HPC):
                                tp = psT.tile([P, P], F32, tag="tp")
                                nc.tensor.transpose(
                                    tp[:].bitcast(F32R),
                                    rp[:, hb * P:(hb + 1) * P], id_sb[:])
                                stg = stgp.tile([P, P], F32R, tag="stg")
                                nc.vector.tensor_copy(stg[:], tp[:])
                                nc.sync.dma_start(tdst[hb, :, rows], stg[:])

                        MG = 1      # m-tiles per xT load
                        for mg in range(MT // MG):
                            grows = slice(mg * MG * P, (mg + 1) * MG * P)
                            xt_sb = xtp.tile([P, KT * MG * P], F32R, tag="xt")
                            nc.sync.dma_start(
                                xt_sb[:].rearrange("p (ko s) -> p ko s", ko=KT),
                                xT[:, grows].rearrange("(ko p) s -> p ko s", p=P))
                            x3 = xt_sb[:].rearrange("p (ko s) -> p ko s", ko=KT)
                            for mi in range(MG):
                                proj_tile(mg * MG + mi, x3, mi)
                    maybe_loop(tc, pass_body)

            # ---- attention + output projection -------------------------
            with tc.tile_pool(name="qt", bufs=2) as qtp, \
                 tc.tile_pool(name="kt", bufs=2) as ktp, \
                 tc.tile_pool(name="vat", bufs=2) as vatp, \
                 tc.tile_pool(name="ex", bufs=18) as exp_, \
                 tc.tile_pool(name="ctx", bufs=1) as ctxp, \
                 tc.tile_pool(name="rec", bufs=3) as recp, \
                 tc.tile_pool(name="wo", bufs=2) as wop, \
                 tc.tile_pool(name="ob", bufs=3) as obp, \
                 tc.tile_pool(name="mt", bufs=3) as mtp, \
                 tc.tile_pool(name="psS", bufs=2, space="PSUM") as psS, \
                 tc.tile_pool(name="psU", bufs=2, space="PSUM") as psU, \
                 tc.tile_pool(name="psC", bufs=2, space="PSUM") as psC, \
                 tc.tile_pool(name="psO", bufs=2, space="PSUM") as psO:

                def attn_body():
                    QCT = S // NT           # 4 q-chunks per batch
                    KTT = S // P            # 16 kp-tiles per batch
                    for b in range(B):
                        ctx_tiles = []
                        for hb in range(HPC):
                            bs0 = b * S
                            qt_sb = qtp.tile([P, S], F32R, tag="qt")
                            nc.sync.dma_start(qt_sb[:], qt_d[hb, :, bs0:bs0 + S])
                            kt_sb = ktp.tile([P, S], F32R, tag="kt")
                            nc.sync.dma_start(kt_sb[:], kt_d[hb, :, bs0:bs0 + S])
                            v_sb = vatp.tile([P, KTT * P], F32R, tag="v")
                            nc.sync.dma_start(
                                v_sb[:].rearrange("p (ko d) -> p ko d", ko=KTT),
                                vnat[bs0:bs0 + S, hb * P:(hb + 1) * P]
                                .rearrange("(ko p) d -> p ko d", p=P))
                            v3 = v_sb[:].rearrange("p (ko d) -> p ko d", ko=KTT)
                            ctx_sb = ctxp.tile([P, S], F32R, tag=f"ctx{hb}")
                            ctx_tiles.append(ctx_sb)
                            for qc in range(QCT):
                                kt_hi = 4 * qc + 4 if mode == "causal" else KTT
                                sum_ps = psU.tile([P, NT], F32, tag="sum")
                                ctx_ps = psC.tile([P, NT], F32, tag="ctx")
                                for k in range(kt_hi):
                                    sc = psS.tile([P, NT], F32, tag="sc")
                                    nc.tensor.matmul(
                                        sc[:], kt_sb[:, k * P:(k + 1) * P],
                                        qt_sb[:, qc * NT:(qc + 1) * NT],
                                        start=True, stop=True)
                                    if mode == "causal" and k >= 4 * qc:
                                        o = k - 4 * qc
                                        nc.vector.tensor_add(
                                            sc[:], sc[:],
                                            md_sb[:, o * NT:(o + 1) * NT])
                                    elif mode == "general":
                                        mt_sb = mtp.tile([P, NT], F32, tag="mt")
                                        nc.sync.dma_start(
                                            mt_sb[:],
                                            maskT[k * P:(k + 1) * P,
                                                  qc * NT:(qc + 1) * NT])
                                        nc.vector.tensor_add(sc[:], sc[:], mt_sb[:])
                                    ex = exp_.tile([P, NT], F32R, tag="ex")
                                    nc.scalar.activation(ex[:], sc[:], EXP,
                                                         scale=SCALE)
                                    nc.tensor.matmul(sum_ps[:], ones_sb[:], ex[:],
                                                     start=(k == 0),
                                                     stop=(k == kt_hi - 1))
                                    nc.tensor.matmul(ctx_ps[:], v3[:, k, :], ex[:],
                                                     start=(k == 0),
                                                     stop=(k == kt_hi - 1))
                                rec = recp.tile([P, NT], F32, tag="rec")
                                nc.vector.reciprocal(rec[:], sum_ps[:])
                                nc.vector.tensor_mul(
                                    ctx_sb[:, qc * NT:(qc + 1) * NT],
                                    ctx_ps[:], rec[:])
                        # wo for batch b
                        wo3_d = wo.rearrange("(ko p) n -> p ko n", p=P)
                        for n in range(DIM // NT):
                            won = wop.tile([P, HPC * NT], F32R, tag="won")
                            nc.sync.dma_start(
                                won[:].rearrange("p (ko n) -> p ko n", ko=HPC),
                                wo3_d[:, :, n * NT:(n + 1) * NT])
                            w3 = won[:].rearrange("p (ko n) -> p ko n", ko=HPC)
                            for m in range(S // P):
                                op = psO.tile([P, NT], F32, tag="op")
                                for h in range(HPC):
                                    nc.tensor.matmul(
                                        op[:],
                                        ctx_tiles[h][:, m * P:(m + 1) * P],
                                        w3[:, h, :],
                                        start=(h == 0), stop=(h == HPC - 1))
                                ob = obp.tile([P, NT], F32, tag="ob")
                                nc.vector.tensor_copy(ob[:], op[:])
                                nc.sync.dma_start(
                                    pout[b * S + m * P: b * S + (m + 1) * P,
                                         n * NT:(n + 1) * NT], ob[:])
                maybe_loop(tc, attn_body)
    nc.finalize()
    return nc


class _Runner:
    """SPMD executor via PJRT/axon, inputs kept on device between runs."""

    def __init__(self, nc, n_cores=N_CORES):
        import jax
        from jax.sharding import Mesh, PartitionSpec, NamedSharding
        from jax.experimental.shard_map import shard_map
        from concourse.bass2jax import (_bass_exec_p, install_neuronx_cc_hook,
                                        partition_id_tensor)
        install_neuronx_cc_hook()
        self.jax = jax
        self.n_cores = n_cores
        pname = nc.partition_id_tensor.name if nc.partition_id_tensor else None
        in_names, out_names, out_avals, zero_outs = [], [], [], []
        for alloc in nc.m.functions[0].allocations:
            if not isinstance(alloc, mybir.MemoryLocationSet):
                continue
            name = alloc.memorylocations[0].name
            if alloc.kind == "ExternalInput":
                if name != pname:
                    in_names.append(name)
            elif alloc.kind == "ExternalOutput":
                out_names.append(name)
                shape = tuple(alloc.tensor_shape)
                dtype = mybir.dt.np(alloc.dtype)
                out_avals.append(jax.core.ShapedArray(shape, dtype))
                zero_outs.append(np.zeros(shape, dtype))
        self.in_names, self.out_names = in_names, out_names
        self.out_avals, self.zero_outs = out_avals, zero_outs
        n_params = len(in_names)
        all_in = in_names + out_names + ([pname] if pname else [])

        def _body(*args):
            operands = list(args)
            if pname is not None:
                operands.append(partition_id_tensor())
            return tuple(_bass_exec_p.bind(
                *operands, out_avals=tuple(out_avals), in_names=tuple(all_in),
                out_names=tuple(out_names), lowering_input_output_aliases=(),
                sim_require_finite=True, sim_require_nnan=True, nc=nc))

        devices = jax.devices()[:n_cores]
        mesh = Mesh(np.asarray(devices), ("core",))
        specs_in = (PartitionSpec("core"),) * (n_params + len(out_names))
        specs_out = (PartitionSpec("core"),) * len(out_names)
        self.fn = jax.jit(
            shard_map(_body, mesh=mesh, in_specs=specs_in,
                      out_specs=specs_out, check_rep=False),
            donate_argnums=tuple(range(n_params, n_params + len(out_names))),
            keep_unused=True)
        self.sharding = NamedSharding(mesh, PartitionSpec("core"))
        self._dev_in = None

    def put_inputs(self, in_maps):
        jax = self.jax
        concat = [np.concatenate([np.asarray(in_maps[c][nm])
                                  for c in range(self.n_cores)], axis=0)
                  for nm in self.in_names]
        self._dev_in = [jax.device_put(a, self.sharding) for a in concat]
        for a in self._dev_in:
            a.block_until_ready()

    def run(self):
        import time
        jax = self.jax
        zs = [jax.device_put(
            np.zeros((self.n_cores * z.shape[0], *z.shape[1:]), z.dtype),
            self.sharding) for z in self.zero_outs]
        for z in zs:
            z.block_until_ready()
        t0 = time.perf_counter()
        outs = self.fn(*self._dev_in, *zs)
        for o in outs:
            o.block_until_ready()
        dt = time.perf_counter() - t0
        res = []
        for c in range(self.n_cores):
            d = {nm: np.asarray(outs[i]).reshape(self.n_cores,
                                                 *self.out_avals[i].shape)[c]
                 for i, nm in enumerate(self.out_names)}
            res.append(d)
        return res, dt


def _detect_mode(mask: np.ndarray) -> str:
    m2 = np.asarray(mask).reshape(S, S)
    if not np.any(m2):
        return "zero"
    causal = np.triu(np.full((S, S), -np.inf, dtype=np.float32), k=1)
    if np.array_equal(m2, causal):
        return "causal"
    return "general"


def _host_prep(x, freqs_cis, mask, mode):
    """Build per-core input maps."""
    xf = np.ascontiguousarray(np.asarray(x, dtype=np.float32).reshape(BS, DIM))
    xT = np.ascontiguousarray(xf.T)
    f = np.asarray(freqs_cis, dtype=np.float32)          # (S, 64, 2)
    cos, sin = f[..., 0], f[..., 1]                       # (S, 64)
    ropeC = np.repeat(cos, 2, axis=1).astype(np.float32)  # (S, 128)
    ropeS = np.empty((S, HEAD_DIM), np.float32)
    ropeS[:, 0::2] = -sin
    ropeS[:, 1::2] = sin
    ident = np.eye(P, dtype=np.float32)
    ones = np.ones((P, P), np.float32)
    extra = {}
    if mode == "causal":
        md = np.zeros((P, 4 * NT), np.float32)
        for o in range(4):
            kp = np.arange(P)[:, None] + o * P
            q = np.arange(NT)[None, :]
            md[:, o * NT:(o + 1) * NT] = np.where(kp > q, -1e4, 0.0)
        extra["maskdiag"] = md
    elif mode == "general":
        m2 = np.asarray(mask, dtype=np.float32).reshape(S, S)
        extra["maskT"] = np.maximum(m2.T / SCALE, -1e4).astype(np.float32)
    wqf = np.asarray(_host_prep.wq, dtype=np.float32)
    wkf = np.asarray(_host_prep.wk, dtype=np.float32)
    wvf = np.asarray(_host_prep.wv, dtype=np.float32)
    wof = np.asarray(_host_prep.wo, dtype=np.float32)
    in_maps = []
    for c in range(N_CORES):
        cols = slice(c * HD_C, (c + 1) * HD_C)
        in_maps.append({
            "xT": xT, "wq": np.ascontiguousarray(wqf[:, cols]),
            "wk": np.ascontiguousarray(wkf[:, cols]),
            "wv": np.ascontiguousarray(wvf[:, cols]),
            "wo": np.ascontiguousarray(wof[cols, :]),
            "ropeC": ropeC, "ropeS": ropeS, "ident": ident, "ones": ones,
            **extra,
        })
    return in_maps


def _get_runner(mode, loop_r=1):
    key = (mode, loop_r)
    if key not in _runner_cache:
        nc = _build(mode, loop_r)
        _runner_cache[key] = _Runner(nc)
    return _runner_cache[key]


def _assemble(results, hidden_state):
    out = np.zeros((BS, DIM), np.float32)
    for c in range(N_CORES):
        out += results[c]["pout"]
    out = out.reshape(B, S, DIM)
    nh = np.array(hidden_state, dtype=np.float32, copy=True)
    for c in range(N_CORES):
        h0 = c * HPC
        k = results[c]["knat"].reshape(B, S, HPC, HEAD_DIM)
        v = results[c]["vnat"].reshape(B, S, HPC, HEAD_DIM)
        nh[0, :B, :, h0:h0 + HPC, :] = k
        nh[1, :B, :, h0:h0 + HPC, :] = v
    return out, nh


def kernel(x, start_pos, freqs_cis, mask, hidden_state, wq, wk, wv, wo):
    assert int(start_pos) == 0, "kernel specialized for start_pos=0 prefill"
    mode = _detect_mode(mask)
    _host_prep.wq, _host_prep.wk, _host_prep.wv, _host_prep.wo = wq, wk, wv, wo
    in_maps = _host_prep(x, freqs_cis, mask, mode)
    runner = _get_runner(mode, loop_r=1)
    runner.put_inputs(in_maps)
    results, _ = runner.run()
    return _assemble(results, hidden_state)


# revision 6
# speedup vs baseline: 3.7228x; 1.3966x over previous
"""Trainium2 Bass kernel for nn_Attention_19430432047265.

Multi-head attention block (B=2, S=2048, D=4096, 32 heads, head_dim=128) with
RoPE, KV-cache update, causal softmax, and output projection — tensor-parallel
over heads across 8 NeuronCores (4 heads/core), wo row-sharded with host-side
reduction of partial outputs.

Per-core dataflow (all matmuls in fp32r — full-rate PE, ~fp32 precision):
  Pass Q/K/V: natural-layout projections from host-pretransposed xT
              (lhsT = xT tile [128k,128s], rhs = w [128k,512]), RoPE applied
              in natural layout via strided DVE ops, Q/K PE-transposed to
              [d, s] layout and spilled to DRAM.
  Attention:  per (batch, head): scoresT[kp,q] = KT_tile.T @ QT (contraction
              over d=128 in one matmul), unstable softmax (exp on ACT with the
              1/sqrt(d) scale folded in; column sums via an all-ones stationary
              matmul which also broadcasts the sums to all 128 partitions;
              masked tiles skipped / diagonal tiles get an additive -1e4 mask
              before exp), ctxT[dv,q] += V_tile.T @ expT accumulated in PSUM,
              then ctxT *= 1/sums.
  Output:     out[s,:] = sum_h ctxT_h[:, s].T @ wo_rows_h — partial over this
              core's heads; host sums the 8 partials.
"""
import sys
sys.path.insert(0, "/opt/trn_rl_repo")
import numpy as np

import concourse.bass as bass
import concourse.mybir as mybir
import concourse.tile as tile
from concourse import bacc

B, S, DIM, N_HEADS, HEAD_DIM = 2, 2048, 4096, 32, 128
BS = B * S                      # 4096 flattened rows
N_CORES = 8
HPC = N_HEADS // N_CORES        # 4 heads per core
HD_C = HPC * HEAD_DIM           # 512 cols per core
P = 128
NT = 512                        # matmul moving free dim
KT = DIM // P                   # 32 k-tiles for projections
MT = BS // P                    # 32 s-tiles
SCALE = float(1.0 / np.sqrt(HEAD_DIM))

F32 = mybir.dt.float32
F32R = mybir.dt.float32r
EXP = mybir.ActivationFunctionType.Exp

_runner_cache: dict = {}


def _build(mode: str, loop_r: int = 1):
    """mode: 'causal' | 'zero' | 'general'."""
    nc = bacc.Bacc("TRN2", target_bir_lowering=False, debug=False,
                   num_devices=N_CORES)
    xT = nc.declare_dram_parameter("xT", [DIM, BS], F32R, isOutput=False)
    wq = nc.declare_dram_parameter("wq", [DIM, HD_C], F32R, isOutput=False)
    wk = nc.declare_dram_parameter("wk", [DIM, HD_C], F32R, isOutput=False)
    wv = nc.declare_dram_parameter("wv", [DIM, HD_C], F32R, isOutput=False)
    wo = nc.declare_dram_parameter("wo", [HD_C, DIM], F32R, isOutput=False)
    ropeC = nc.declare_dram_parameter("ropeC", [S, HEAD_DIM], F32, isOutput=False)
    ropeS = nc.declare_dram_parameter("ropeS", [S, HEAD_DIM], F32, isOutput=False)
    ident = nc.declare_dram_parameter("ident", [P, P], F32R, isOutput=False)
    ones = nc.declare_dram_parameter("ones", [P, P], F32R, isOutput=False)
    if mode == "causal":
        maskdiag = nc.declare_dram_parameter("maskdiag", [P, 4 * NT], F32,
                                             isOutput=False)
    elif mode == "general":
        maskT = nc.declare_dram_parameter("maskT", [S, S], F32, isOutput=False)
    knat = nc.declare_dram_parameter("knat", [BS, HD_C], F32R, isOutput=True)
    vnat = nc.declare_dram_parameter("vnat", [BS, HD_C], F32R, isOutput=True)
    pout = nc.declare_dram_parameter("pout", [BS, DIM], F32, isOutput=True)
    qt_d = nc.dram_tensor("qt_d", [HPC, P, BS], F32R)
    kt_d = nc.dram_tensor("kt_d", [HPC, P, BS], F32R)

    def maybe_loop(tc, fn):
        if loop_r > 1:
            with tc.For_i(0, loop_r, 1):
                fn()
        else:
            fn()

    with tile.TileContext(nc) as tc:
        with tc.tile_pool(name="const", bufs=1) as constp:
            id_sb = constp.tile([P, P], F32R, tag="id")
            nc.sync.dma_start(id_sb[:], ident[:])
            ones_sb = constp.tile([P, P], F32R, tag="ones")
            nc.sync.dma_start(ones_sb[:], ones[:])
            if mode == "causal":
                md_sb = constp.tile([P, 4 * NT], F32, tag="md")
                nc.sync.dma_start(md_sb[:], maskdiag[:])
            STP = S // P
            rc_sb = constp.tile([P, STP * HEAD_DIM], F32, tag="rc")
            nc.sync.dma_start(
                rc_sb[:].rearrange("p (st d) -> p st d", st=STP),
                ropeC.rearrange("(st p) d -> p st d", p=P))
            rs_sb = constp.tile([P, STP * HEAD_DIM], F32, tag="rs")
            nc.sync.dma_start(
                rs_sb[:].rearrange("p (st d) -> p st d", st=STP),
                ropeS.rearrange("(st p) d -> p st d", p=P))
            rc3 = rc_sb[:].rearrange("p (st d) -> p st d", st=STP)
            rs3 = rs_sb[:].rearrange("p (st d) -> p st d", st=STP)

            # ---- projection passes -------------------------------------
            for pname, w_h, tdst, out_nat in (
                    ("q", wq, qt_d, None),
                    ("k", wk, kt_d, knat),
                    ("v", wv, None, vnat)):
                with tc.tile_pool(name=f"w_{pname}", bufs=1) as wp, \
                     tc.tile_pool(name=f"xt_{pname}", bufs=2) as xtp, \
                     tc.tile_pool(name=f"rp_{pname}", bufs=6) as rpp, \
                     tc.tile_pool(name=f"stg_{pname}", bufs=6) as stgp, \
                     tc.tile_pool(name=f"psA_{pname}", bufs=2, space="PSUM") as psA, \
                     tc.tile_pool(name=f"psT_{pname}", bufs=2, space="PSUM") as psT:

                    def pass_body(pname=pname, w_h=w_h, tdst=tdst,
                                  out_nat=out_nat, wp=wp, xtp=xtp,
                                  rpp=rpp, stgp=stgp, psA=psA, psT=psT):
                        w_sb = wp.tile([P, KT * NT], F32R, tag="w")
                        nc.sync.dma_start(
                            w_sb[:].rearrange("p (ko n) -> p ko n", ko=KT),
                            w_h.rearrange("(ko p) n -> p ko n", p=P))
                        w3 = w_sb[:].rearrange("p (ko n) -> p ko n", ko=KT)

                        def proj_tile(m, x3, mi):
                            rows = slice(m * P, (m + 1) * P)
                            ps = psA.tile([P, NT], F32, tag="ps")
                            for k in range(KT):
                                nc.tensor.matmul(
                                    ps[:], x3[:, k, mi * P:(mi + 1) * P],
                                    w3[:, k, :],
                                    start=(k == 0), stop=(k == KT - 1))
                            if pname == "v":
                                vs = rpp.tile([P, NT], F32R, tag="ev")
                                nc.vector.tensor_copy(vs[:], ps[:])
                                nc.scalar.dma_start(out_nat[rows, :], vs[:])
                                return
                            # RoPE in natural layout (resident tables)
                            st = m % (S // P)
                            c_sb = rc3[:, st, :]
                            s_sb = rs3[:, st, :]
                            ps3 = ps[:].rearrange("p (hb d) -> p hb d", hb=HPC)
                            sw = rpp.tile([P, NT], F32, tag="sw")
                            sw3 = sw[:].rearrange("p (hb d) -> p hb d", hb=HPC)
                            nc.vector.tensor_copy(sw3[:, :, 0::2], ps3[:, :, 1::2])
                            nc.vector.tensor_copy(sw3[:, :, 1::2], ps3[:, :, 0::2])
                            cb = c_sb[:, None, :].broadcast_to((P, HPC, HEAD_DIM))
                            sb_ = s_sb[:, None, :].broadcast_to((P, HPC, HEAD_DIM))
                            tmp = rpp.tile([P, NT], F32, tag="tmp")
                            tmp3 = tmp[:].rearrange("p (hb d) -> p hb d", hb=HPC)
                            nc.vector.tensor_mul(tmp3[:], ps3[:], cb)
                            nc.vector.tensor_mul(sw3[:], sw3[:], sb_)
                            rp = rpp.tile([P, NT], F32R, tag="rp")
                            nc.vector.tensor_add(rp[:], tmp[:], sw[:])
                            if out_nat is not None:
                                nc.scalar.dma_start(out_nat[rows, :], rp[:])
                            stg = stgp.tile([P, HPC * P], F32R, tag="stg")
                            for hb in range(HPC):
                                tp = psT.tile([P, P], F32, tag="tp")
                                nc.tensor.transpose(
                                    tp[:].bitcast(F32R),
                                    rp[:, hb * P:(hb + 1) * P], id_sb[:])
                                nc.vector.tensor_copy(
                                    stg[:, hb * P:(hb + 1) * P], tp[:])
                            nc.scalar.dma_start(
                                tdst[:, :, rows].rearrange("h p s -> p h s"),
                                stg[:].rearrange("p (h s) -> p h s", h=HPC))

                        MG = 1      # m-tiles per xT load
                        for mg in range(MT // MG):
                            grows = slice(mg * MG * P, (mg + 1) * MG * P)
                            xt_sb = xtp.tile([P, KT * MG * P], F32R, tag="xt")
                            nc.sync.dma_start(
                                xt_sb[:].rearrange("p (ko s) -> p ko s", ko=KT),
                                xT[:, grows].rearrange("(ko p) s -> p ko s", p=P))
                            x3 = xt_sb[:].rearrange("p (ko s) -> p ko s", ko=KT)
                            for mi in range(MG):
                                proj_tile(mg * MG + mi, x3, mi)
                    maybe_loop(tc, pass_body)

            # ---- attention + output projection -------------------------
            with tc.tile_pool(name="qt", bufs=2) as qtp, \
                 tc.tile_pool(name="kt", bufs=2) as ktp, \
                 tc.tile_pool(name="vat", bufs=2) as vatp, \
                 tc.tile_pool(name="ex", bufs=18) as exp_, \
                 tc.tile_pool(name="ctx", bufs=1) as ctxp, \
                 tc.tile_pool(name="rec", bufs=3) as recp, \
                 tc.tile_pool(name="wo", bufs=2) as wop, \
                 tc.tile_pool(name="ob", bufs=3) as obp, \
                 tc.tile_pool(name="mt", bufs=3) as mtp, \
                 tc.tile_pool(name="psS", bufs=2, space="PSUM") as psS, \
                 tc.tile_pool(name="psU", bufs=2, space="PSUM") as psU, \
                 tc.tile_pool(name="psC", bufs=2, space="PSUM") as psC, \
                 tc.tile_pool(name="psO", bufs=2, space="PSUM") as psO:

                def attn_body():
                    QCT = S // NT           # 4 q-chunks per batch
                    KTT = S // P            # 16 kp-tiles per batch
                    for b in range(B):
                        ctx_tiles = []
                        for hb in range(HPC):
                            bs0 = b * S
                            qt_sb = qtp.tile([P, S], F32R, tag="qt")
                            nc.sync.dma_start(qt_sb[:], qt_d[hb, :, bs0:bs0 + S])
                            kt_sb = ktp.tile([P, S], F32R, tag="kt")
                            nc.sync.dma_start(kt_sb[:], kt_d[hb, :, bs0:bs0 + S])
                            v_sb = vatp.tile([P, KTT * P], F32R, tag="v")
                            nc.sync.dma_start(
                                v_sb[:].rearrange("p (ko d) -> p ko d", ko=KTT),
                                vnat[bs0:bs0 + S, hb * P:(hb + 1) * P]
                                .rearrange("(ko p) d -> p ko d", p=P))
                            v3 = v_sb[:].rearrange("p (ko d) -> p ko d", ko=KTT)
                            ctx_sb = ctxp.tile([P, S], F32R, tag=f"ctx{hb}")
                            ctx_tiles.append(ctx_sb)
                            for qc in range(QCT):
                                kt_hi = 4 * qc + 4 if mode == "causal" else KTT
                                sum_ps = psU.tile([P, NT], F32, tag="sum")
                                ctx_ps = psC.tile([P, NT], F32, tag="ctx")
                                for k in range(kt_hi):
                                    sc = psS.tile([P, NT], F32, tag="sc")
                                    nc.tensor.matmul(
                                        sc[:], kt_sb[:, k * P:(k + 1) * P],
                                        qt_sb[:, qc * NT:(qc + 1) * NT],
                                        start=True, stop=True)
                                    if mode == "causal" and k >= 4 * qc:
                                        o = k - 4 * qc
                                        nc.vector.tensor_add(
                                            sc[:], sc[:],
                                            md_sb[:, o * NT:(o + 1) * NT])
                                    elif mode == "general":
                                        mt_sb = mtp.tile([P, NT], F32, tag="mt")
                                        nc.sync.dma_start(
                                            mt_sb[:],
                                            maskT[k * P:(k + 1) * P,
                                                  qc * NT:(qc + 1) * NT])
                                        nc.vector.tensor_add(sc[:], sc[:], mt_sb[:])
                                    ex = exp_.tile([P, NT], F32R, tag="ex")
                                    nc.scalar.activation(ex[:], sc[:], EXP,
                                                         scale=SCALE)
                                    nc.tensor.matmul(sum_ps[:], ones_sb[:], ex[:],
                                                     start=(k == 0),
                                                     stop=(k == kt_hi - 1))
                                    nc.tensor.matmul(ctx_ps[:], v3[:, k, :], ex[:],
                                                     start=(k == 0),
                                                     stop=(k == kt_hi - 1))
                                rec = recp.tile([P, NT], F32, tag="rec")
                                nc.vector.reciprocal(rec[:], sum_ps[:])
                                nc.vector.tensor_mul(
                                    ctx_sb[:, qc * NT:(qc + 1) * NT],
                                    ctx_ps[:], rec[:])
                        # wo for batch b
                        wo3_d = wo.rearrange("(ko p) n -> p ko n", p=P)
                        for n in range(DIM // NT):
                            won = wop.tile([P, HPC * NT], F32R, tag="won")
                            nc.sync.dma_start(
                                won[:].rearrange("p (ko n) -> p ko n", ko=HPC),
                                wo3_d[:, :, n * NT:(n + 1) * NT])
                            w3 = won[:].rearrange("p (ko n) -> p ko n", ko=HPC)
                            MGO = 4
                            for mg in range(S // P // MGO):
                                ob4 = obp.tile([P, MGO * NT], F32, tag="ob")
                                for mi in range(MGO):
                                    m = mg * MGO + mi
                                    op = psO.tile([P, NT], F32, tag="op")
                                    for h in range(HPC):
                                        nc.tensor.matmul(
                                            op[:],
                                            ctx_tiles[h][:, m * P:(m + 1) * P],
                                            w3[:, h, :],
                                            start=(h == 0), stop=(h == HPC - 1))
                                    nc.vector.tensor_copy(
                                        ob4[:, mi * NT:(mi + 1) * NT], op[:])
                                r0 = b * S + mg * MGO * P
                                nc.scalar.dma_start(
                                    pout[r0:r0 + MGO * P, n * NT:(n + 1) * NT]
                                    .rearrange("(m p) n -> p m n", p=P),
                                    ob4[:].rearrange("p (m n) -> p m n", m=MGO))
                maybe_loop(tc, attn_body)
    nc.finalize()
    return nc


class _Runner:
    """SPMD executor via PJRT/axon, inputs kept on device between runs."""

    def __init__(self, nc, n_cores=N_CORES):
        import jax
        from jax.sharding import Mesh, PartitionSpec, NamedSharding
        from jax.experimental.shard_map import shard_map
        from concourse.bass2jax import (_bass_exec_p, install_neuronx_cc_hook,
                                        partition_id_tensor)
        install_neuronx_cc_hook()
        self.jax = jax
        self.n_cores = n_cores
        pname = nc.partition_id_tensor.name if nc.partition_id_tensor else None
        in_names, out_names, out_avals, zero_outs = [], [], [], []
        for alloc in nc.m.functions[0].allocations:
            if not isinstance(alloc, mybir.MemoryLocationSet):
                continue
            name = alloc.memorylocations[0].name
            if alloc.kind == "ExternalInput":
                if name != pname:
                    in_names.append(name)
            elif alloc.kind == "ExternalOutput":
                out_names.append(name)
                shape = tuple(alloc.tensor_shape)
                dtype = mybir.dt.np(alloc.dtype)
                out_avals.append(jax.core.ShapedArray(shape, dtype))
                zero_outs.append(np.zeros(shape, dtype))
        self.in_names, self.out_names = in_names, out_names
        self.out_avals, self.zero_outs = out_avals, zero_outs
        n_params = len(in_names)
        all_in = in_names + out_names + ([pname] if pname else [])

        def _body(*args):
            operands = list(args)
            if pname is not None:
                operands.append(partition_id_tensor())
            return tuple(_bass_exec_p.bind(
                *operands, out_avals=tuple(out_avals), in_names=tuple(all_in),
                out_names=tuple(out_names), lowering_input_output_aliases=(),
                sim_require_finite=True, sim_require_nnan=True, nc=nc))

        devices = jax.devices()[:n_cores]
        mesh = Mesh(np.asarray(devices), ("core",))
        specs_in = (PartitionSpec("core"),) * (n_params + len(out_names))
        specs_out = (PartitionSpec("core"),) * len(out_names)
        self.fn = jax.jit(
            shard_map(_body, mesh=mesh, in_specs=specs_in,
                      out_specs=specs_out, check_rep=False),
            donate_argnums=tuple(range(n_params, n_params + len(out_names))),
            keep_unused=True)
        self.sharding = NamedSharding(mesh, PartitionSpec("core"))
        self._dev_in = None

    def put_inputs(self, in_maps):
        jax = self.jax
        concat = [np.concatenate([np.asarray(in_maps[c][nm])
                                  for c in range(self.n_cores)], axis=0)
                  for nm in self.in_names]
        self._dev_in = [jax.device_put(a, self.sharding) for a in concat]
        for a in self._dev_in:
            a.block_until_ready()

    def run(self):
        import time
        jax = self.jax
        zs = [jax.device_put(
            np.zeros((self.n_cores * z.shape[0], *z.shape[1:]), z.dtype),
            self.sharding) for z in self.zero_outs]
        for z in zs:
            z.block_until_ready()
        t0 = time.perf_counter()
        outs = self.fn(*self._dev_in, *zs)
        for o in outs:
            o.block_until_ready()
        dt = time.perf_counter() - t0
        res = []
        for c in range(self.n_cores):
            d = {nm: np.asarray(outs[i]).reshape(self.n_cores,
                                                 *self.out_avals[i].shape)[c]
                 for i, nm in enumerate(self.out_names)}
            res.append(d)
        return res, dt


def _detect_mode(mask: np.ndarray) -> str:
    m2 = np.asarray(mask).reshape(S, S)
    if not np.any(m2):
        return "zero"
    causal = np.triu(np.full((S, S), -np.inf, dtype=np.float32), k=1)
    if np.array_equal(m2, causal):
        return "causal"
    return "general"


def _host_prep(x, freqs_cis, mask, mode):
    """Build per-core input maps."""
    xf = np.ascontiguousarray(np.asarray(x, dtype=np.float32).reshape(BS, DIM))
    xT = np.ascontiguousarray(xf.T)
    f = np.asarray(freqs_cis, dtype=np.float32)          # (S, 64, 2)
    cos, sin = f[..., 0], f[..., 1]                       # (S, 64)
    ropeC = np.repeat(cos, 2, axis=1).astype(np.float32)  # (S, 128)
    ropeS = np.empty((S, HEAD_DIM), np.float32)
    ropeS[:, 0::2] = -sin
    ropeS[:, 1::2] = sin
    ident = np.eye(P, dtype=np.float32)
    ones = np.ones((P, P), np.float32)
    extra = {}
    if mode == "causal":
        md = np.zeros((P, 4 * NT), np.float32)
        for o in range(4):
            kp = np.arange(P)[:, None] + o * P
            q = np.arange(NT)[None, :]
            md[:, o * NT:(o + 1) * NT] = np.where(kp > q, -1e4, 0.0)
        extra["maskdiag"] = md
    elif mode == "general":
        m2 = np.asarray(mask, dtype=np.float32).reshape(S, S)
        extra["maskT"] = np.maximum(m2.T / SCALE, -1e4).astype(np.float32)
    wqf = np.asarray(_host_prep.wq, dtype=np.float32)
    wkf = np.asarray(_host_prep.wk, dtype=np.float32)
    wvf = np.asarray(_host_prep.wv, dtype=np.float32)
    wof = np.asarray(_host_prep.wo, dtype=np.float32)
    in_maps = []
    for c in range(N_CORES):
        cols = slice(c * HD_C, (c + 1) * HD_C)
        in_maps.append({
            "xT": xT, "wq": np.ascontiguousarray(wqf[:, cols]),
            "wk": np.ascontiguousarray(wkf[:, cols]),
            "wv": np.ascontiguousarray(wvf[:, cols]),
            "wo": np.ascontiguousarray(wof[cols, :]),
            "ropeC": ropeC, "ropeS": ropeS, "ident": ident, "ones": ones,
            **extra,
        })
    return in_maps


def _get_runner(mode, loop_r=1):
    key = (mode, loop_r)
    if key not in _runner_cache:
        nc = _build(mode, loop_r)
        _runner_cache[key] = _Runner(nc)
    return _runner_cache[key]


def _assemble(results, hidden_state):
    out = np.zeros((BS, DIM), np.float32)
    for c in range(N_CORES):
        out += results[c]["pout"]
    out = out.reshape(B, S, DIM)
    nh = np.array(hidden_state, dtype=np.float32, copy=True)
    for c in range(N_CORES):
        h0 = c * HPC
        k = results[c]["knat"].reshape(B, S, HPC, HEAD_DIM)
        v = results[c]["vnat"].reshape(B, S, HPC, HEAD_DIM)
        nh[0, :B, :, h0:h0 + HPC, :] = k
        nh[1, :B, :, h0:h0 + HPC, :] = v
    return out, nh


def kernel(x, start_pos, freqs_cis, mask, hidden_state, wq, wk, wv, wo):
    assert int(start_pos) == 0, "kernel specialized for start_pos=0 prefill"
    mode = _detect_mode(mask)
    _host_prep.wq, _host_prep.wk, _host_prep.wv, _host_prep.wo = wq, wk, wv, wo
    in_maps = _host_prep(x, freqs_cis, mask, mode)
    runner = _get_runner(mode, loop_r=1)
    runner.put_inputs(in_maps)
    results, _ = runner.run()
    return _assemble(results, hidden_state)
